# revision 36
# baseline (speedup 1.0000x reference)
"""Trainium2 Bass kernel for nn_CANN_39994735460546.

Reference semantics:
  t    = (physical_params[:, :, None] ** PS_POWERS).reshape(B, 64)
  norm = (t - t.mean()) / t.std(ddof=1)          # global scalar stats
  h    = relu(norm) @ W1.T + b1
  c    = h @ W2.T + b2                            # [B, 5]
  dy[b, j] = sum_k c[b,k] * p_k * eta[b,j]^(p_k - 1),  p = [2,5,8,11,14]

Device strategy (8 NeuronCores, pure data parallel over eta rows; each core
owns 512 rows; stage 1 replicated on every core so no collectives needed):

  Stage 1 (tiny, [B,4] -> per-row poly coefficients):
  - ln(params) computed on host (it is an input-only transform), shipped as
    "pT" [4, B], rolled per core so its own rows come first.
  - t64 = exp(rm.T @ lnp) chunks on PE+ACT; the global stats ride for free
    on the exps' accum_out (a second exp with scale=2 gives sum(t^2)).
  - Coefficients come straight out of one [65,128]x[65,5] matmul per
    128-row block (ones row folds the bias in) -> ctiles [128, 5], f32.

  Stage 2 (the heavy part, dy = eta * P(eta^3) with per-row coefficients),
  spread across ALL engines:
  - ACT: s = fp16(Square(eta)); eh = fp16(eta) (Copy). Square/Copy are
    filler functions present in every activation table set -> ZERO table
    reloads (Ln/Exp-based alternatives pay ~1.3us switches per tile).
  - DVE: u = fp16(s*eh) and the full Horner chain. Since
    scalar_tensor_tensor only has a 1x micro-op, Horner is alternating
    tensor_scalar (4x) / tensor_tensor (2x) steps - 8 cheap 16-bit ops
    beat 5 ops of which 3 run at 1x. Coefficient scalars stay f32
    [128,1], which the perf-mode check permits.
  - No GPSIMD, no mid-chain cross-engine hops: measured on hardware,
    gpsimd elementwise ops pay ~5-20us per-op software overheads, and
    ACT round-trips inside the Horner chain stall the in-order DVE
    queue. (DS below can re-enable a Pool stripe for experiments.)
  - dy stored as bf16 (upcast to f32 on host), halving store traffic.
  - Loads ride the SP HWDGE queue; stores ride ACT, emitted one tile late
    so the ACT queue never stalls waiting on DVE.
"""

import sys
import numpy as np

sys.path.insert(0, "/opt/trn_rl_repo")

B = 4096
L = 4096
NCORES = 8
RPC = B // NCORES          # rows per core = 512
NPT = RPC // 128           # 128-row blocks per core = 4
CT = 4096                  # row width
NDT = 2                    # 128-row blocks per stage-2 double-tile
# DVE Horner stripe width per 4096-col tile. DS=CT disables the GPSIMD
# stripe entirely: measured on hardware, gpsimd elementwise ops carry
# ~5-20us per-op overheads (Q7 software path) that CoreSim's cost model
# does not charge - a 768-col Pool stripe cost 227us/rep vs 62us without.
DS = 4096
UNROLL = 4                 # stage-2 passes per hardware-loop iteration
NTOT = float(B * 64)       # elements in t for the global stats

PS_POWERS = np.array([-5.0, -4.0, -3.0, -2.0, -1.5, -1.0, -0.5, 0.0,
                      0.5, 2.0, 1.0 / 3.0, 3.0, 0.25, 4.0, 0.2, 5.0],
                     dtype=np.float32)
POLY_POWERS = np.array([2.0, 5.0, 8.0, 11.0, 14.0], dtype=np.float32)

_cache = {}


def _build_nc(repeat=1, force_unroll=False):
    import concourse.bass as bass
    import concourse.tile as tile
    from concourse import bacc, mybir

    F32 = mybir.dt.float32
    BF16 = mybir.dt.bfloat16
    F16 = mybir.dt.float16
    AF = mybir.ActivationFunctionType
    OP = mybir.AluOpType
    AX = mybir.AxisListType
    ts = bass.ts

    k1 = 1.0 / (NTOT - 1.0)
    k2 = 1.0 / (NTOT * (NTOT - 1.0))

    nc = bacc.Bacc("TRN2", target_bir_lowering=False, debug=False,
                   num_devices=NCORES)

    eta_d = nc.dram_tensor("eta", [RPC, L], F32, kind="ExternalInput").ap()
    pT_d = nc.dram_tensor("pT", [4, B], F32, kind="ExternalInput").ap()
    rm_d = nc.dram_tensor("rm", [4, 64], F32, kind="ExternalInput").ap()
    wpT_d = nc.dram_tensor("wpT", [65, 5], F32, kind="ExternalInput").ap()
    ones64_d = nc.dram_tensor("ones64", [64, 1], F32, kind="ExternalInput").ap()
    onesr_d = nc.dram_tensor("onesr", [1, 64], F32, kind="ExternalInput").ap()
    dy_d = nc.dram_tensor("dy", [RPC, L], BF16, kind="ExternalOutput").ap()

    from contextlib import ExitStack

    with tile.TileContext(nc) as tc, ExitStack() as stack:
        if True:
            p_const = stack.enter_context(tc.tile_pool(name="consts", bufs=1))
            p_pss = stack.enter_context(
                tc.tile_pool(name="ps_small", bufs=1, space="PSUM"))
            p_psr = stack.enter_context(
                tc.tile_pool(name="ps_r", bufs=2, space="PSUM"))
            p_psc = stack.enter_context(
                tc.tile_pool(name="ps_c", bufs=2, space="PSUM"))
            # ---- constants (rm on SP first: the matmuls wait on it; the
            # rest ride the ACT ring so they don't delay pT/eta) ----------
            rm_sb = p_const.tile([4, 64], F32, tag="rm")
            nc.sync.dma_start(rm_sb[:], rm_d)
            wpT_sb = p_const.tile([65, 5], F32, tag="wpT")
            nc.scalar.dma_start(wpT_sb[:], wpT_d)
            ones64_sb = p_const.tile([64, 1], F32, tag="ones64")
            nc.scalar.dma_start(ones64_sb[:], ones64_d)
            onesr_sb = p_const.tile([1, 64], F32, tag="onesr")
            nc.scalar.dma_start(onesr_sb[:], onesr_d)
            ctiles = [p_const.tile([128, 5], F32, tag=f"ct{t}",
                                   name=f"ct{t}") for t in range(NPT)]

            # Stage-2 pools open BEFORE the stage-1 scratch pool: the stack
            # allocator then gives them disjoint SBUF regions, so the first
            # eta loads don't serialize behind stage-1 reads of recycled
            # addresses.
            p_eta = stack.enter_context(tc.tile_pool(name="eta", bufs=4))
            p_s = stack.enter_context(tc.tile_pool(name="s", bufs=2))
            p_eb = stack.enter_context(tc.tile_pool(name="eb", bufs=2))
            p_u = stack.enter_context(tc.tile_pool(name="u", bufs=2))
            p_g = stack.enter_context(tc.tile_pool(name="g", bufs=2))

            # ---- stage 1 in its own (stack-freed) scratch pool ----
            with (
                tc.tile_pool(name="s1", bufs=1) as p_s1,
                tc.tile_pool(name="s1scr", bufs=2) as p_scr,
            ):
                # pT rides the SP queue FIRST and in 8 chunks: it heads the
                # stage-1 critical path (matmul j waits only on chunk j)
                pT_sb = p_s1.tile([4, B], F32, tag="pT")
                for j in range(B // 512):
                    nc.sync.dma_start(pT_sb[:, ts(j, 512)],
                                      pT_d[:, ts(j, 512)])
                # town rows 0..63: exp chunk for own rows (chunk 0 after the
                # per-core roll); row 64: ones (folds the MLP bias in).
                town = p_s1.tile([65, 512], F32, tag="town")
                acc = p_s1.tile([64, 16], F32, tag="acc")
                sq = p_s1.tile([64, 512], F32, tag="sq")
                nc.vector.memset(town[64:65, :], 1.0)

                for j in range(B // 512):
                    ps_r = p_psr.tile([64, 512], F32, tag="ps_r")
                    nc.tensor.matmul(ps_r[:], rm_sb[:], pT_sb[:, ts(j, 512)],
                                     start=True, stop=True)
                    if j == 0:
                        out_t = town[0:64, :]
                    else:
                        scr_t = p_scr.tile([64, 512], F32, tag="scr",
                                           name=f"scr{j}")
                        out_t = scr_t[:]
                    nc.scalar.activation(out_t, ps_r[:], AF.Exp,
                                         accum_out=acc[:, j:j + 1])
                    # S2 rides DVE (sum of t^2 via accum_out) so the ACT
                    # queue only runs 8 exps, not 16
                    nc.vector.scalar_tensor_tensor(
                        sq[:], out_t, 1.0, out_t, OP.mult, OP.mult,
                        accum_out=acc[:, 8 + j:9 + j])

                # s12[:,0] = sum_j S1 chunks, s12[:,1] = sum_j S2 chunks
                s12 = p_s1.tile([64, 2], F32, tag="s12")
                nc.vector.tensor_reduce(s12[:, 0:1], acc[:, 0:8], AX.X, OP.add)
                nc.vector.tensor_reduce(s12[:, 1:2], acc[:, 8:16], AX.X, OP.add)

                # cross-partition: [1,2] = ones64.T @ s12
                ps_s = p_pss.tile([1, 2], F32, tag="ps_s")
                nc.tensor.matmul(ps_s[:], ones64_sb[:], s12[:],
                                 start=True, stop=True)
                s12sb = p_s1.tile([1, 2], F32, tag="s12sb")
                nc.vector.tensor_copy(s12sb[:], ps_s[:])

                # var = S2/(N-1) - S1^2/(N(N-1)); inv_std = exp(-0.5 ln var)
                scrs = p_s1.tile([1, 4], F32, tag="scrs")
                ab = p_s1.tile([1, 2], F32, tag="ab")
                nc.vector.tensor_scalar(scrs[:, 0:1], s12sb[:, 0:1],
                                        s12sb[:, 0:1], -k2, OP.mult, OP.mult)
                nc.vector.scalar_tensor_tensor(scrs[:, 1:2], s12sb[:, 1:2],
                                               k1, scrs[:, 0:1],
                                               OP.mult, OP.add)
                nc.scalar.activation(scrs[:, 2:3], scrs[:, 1:2], AF.Ln)
                nc.scalar.activation(ab[:, 0:1], scrs[:, 2:3], AF.Exp,
                                     scale=-0.5)
                nc.vector.scalar_tensor_tensor(ab[:, 1:2], s12sb[:, 0:1],
                                               -1.0 / NTOT, ab[:, 0:1],
                                               OP.mult, OP.mult)

                # broadcast (inv_std, bias) to 64 partitions via ones matmul
                ps_b = p_pss.tile([64, 2], F32, tag="ps_b")
                nc.tensor.matmul(ps_b[:], onesr_sb[:], ab[:],
                                 start=True, stop=True)
                ab64 = p_s1.tile([64, 2], F32, tag="ab64")
                nc.vector.tensor_copy(ab64[:], ps_b[:])

                # rn = relu(inv_std * t + bias), in place on town rows 0..63
                nc.scalar.activation(town[0:64, :], town[0:64, :], AF.Relu,
                                     scale=ab64[:, 0:1], bias=ab64[:, 1:2])

                # per 128-row block: ctile [128,5] = town_blk.T @ wpT
                # (row 64 of town is ones -> adds the bias row of wpT)
                for t in range(NPT):
                    ps_c = p_psc.tile([128, 5], F32, tag="ps_c")
                    nc.tensor.matmul(ps_c[:], town[:, ts(t, 128)], wpT_sb[:],
                                     start=True, stop=True)
                    nc.vector.tensor_copy(ctiles[t][:], ps_c[:])

            # ---- stage 2: dy = eta * P(eta^3) ----
            state = {"pending": None}  # store delayed one tile

            def one_pass():
                for t in range(NPT):
                    rows = slice(t * 128, (t + 1) * 128)
                    eta_t = p_eta.tile([128, CT], F32, tag="eta",
                                       name="eta_t")
                    nc.sync.dma_start(eta_t[:], eta_d[rows, :])

                    s_t = p_s.tile([128, CT], F16, tag="s", name="s_t")
                    nc.scalar.activation(s_t[:], eta_t[:], AF.Square)
                    eh_t = p_eb.tile([128, CT], F16, tag="eh", name="eh_t")
                    nc.scalar.activation(eh_t[:], eta_t[:], AF.Copy)
                    if state["pending"] is not None:
                        nc.scalar.dma_start(*state["pending"])
                    u_t = p_u.tile([128, CT], F16, tag="u", name="u_t")
                    g_t = p_g.tile([128, CT], BF16, tag="g", name="g_t")
                    cs = ctiles[t]
                    c0, c1, c2, c3, c4 = (cs[:, k:k + 1] for k in range(5))
                    # DVE stripe [0, DS) and Pool stripe [DS, CT);
                    # each computes its own cube and Horner chain
                    dv = slice(0, DS)
                    g_, u_, eh_ = g_t[:, dv], u_t[:, dv], eh_t[:, dv]
                    nc.vector.tensor_tensor(u_, s_t[:, dv], eh_, OP.mult)
                    nc.vector.tensor_scalar(g_, u_, c4, c3, OP.mult, OP.add)
                    nc.vector.tensor_tensor(g_, g_, u_, OP.mult)
                    nc.vector.tensor_scalar(g_, g_, c2, None, OP.add)
                    nc.vector.tensor_tensor(g_, g_, u_, OP.mult)
                    nc.vector.tensor_scalar(g_, g_, c1, None, OP.add)
                    nc.vector.tensor_tensor(g_, g_, u_, OP.mult)
                    nc.vector.tensor_scalar(g_, g_, c0, None, OP.add)
                    nc.vector.tensor_tensor(g_, g_, eh_, OP.mult)

                    if DS < CT:
                        pl = slice(DS, CT)
                        gp, up, ep = g_t[:, pl], u_t[:, pl], eh_t[:, pl]
                        nc.gpsimd.tensor_tensor(up, s_t[:, pl], ep, OP.mult)
                        nc.gpsimd.tensor_scalar(gp, up, c4, c3,
                                                OP.mult, OP.add)
                        nc.gpsimd.tensor_tensor(gp, gp, up, OP.mult)
                        nc.gpsimd.tensor_scalar(gp, gp, c2, None, OP.add)
                        nc.gpsimd.tensor_tensor(gp, gp, up, OP.mult)
                        nc.gpsimd.tensor_scalar(gp, gp, c1, None, OP.add)
                        nc.gpsimd.tensor_tensor(gp, gp, up, OP.mult)
                        nc.gpsimd.tensor_scalar(gp, gp, c0, None, OP.add)
                        nc.gpsimd.tensor_tensor(gp, gp, ep, OP.mult)
                    state["pending"] = (dy_d[rows, :], g_t[:])

            def flush():
                if state["pending"] is not None:
                    nc.scalar.dma_start(*state["pending"])
                    state["pending"] = None

            if repeat <= UNROLL or force_unroll:
                for _ in range(repeat):
                    one_pass()
                flush()
            else:
                # hardware loop: constant NEFF size for any repeat count, so
                # huge repeats amplify the timing signal above the multi-
                # second axon dispatch noise. UNROLL passes per iteration
                # amortize the per-iteration all-engine barrier.
                n_iter, rem = divmod(repeat, UNROLL)
                with tc.For_i(0, n_iter):
                    for _ in range(UNROLL):
                        one_pass()
                    flush()
                for _ in range(rem):
                    one_pass()
                flush()
    nc.compile()
    return nc


def _host_prep(physical_params, W1, b1, W2, b2):
    pp = np.ascontiguousarray(physical_params, dtype=np.float32)
    W1 = np.asarray(W1, dtype=np.float32)
    b1 = np.asarray(b1, dtype=np.float32)
    W2 = np.asarray(W2, dtype=np.float32)
    b2 = np.asarray(b2, dtype=np.float32)

    # fused MLP (no activation between the linears) + fold p_k
    Weff = W2 @ W1                       # [5, 64]
    beff = W2 @ b1 + b2                  # [5]
    Wp = POLY_POWERS[:, None] * Weff     # [5, 64]
    bp = POLY_POWERS * beff              # [5]

    # [65, 5]: MLP weights with the bias as a final row (ones-row trick)
    wpT = np.concatenate([Wp.T, bp[None, :]], axis=0)

    # replication+scale matrix: rm[i, i*16+j] = PS_POWERS[j]
    rm = np.zeros((4, 64), np.float32)
    for i in range(4):
        rm[i, i * 16:(i + 1) * 16] = PS_POWERS

    consts = {
        "rm": rm,
        "wpT": np.ascontiguousarray(wpT, dtype=np.float32),
        "ones64": np.ones((64, 1), np.float32),
        "onesr": np.ones((1, 64), np.float32),
    }
    # ln on host: pT carries ln(params).T
    return np.ascontiguousarray(np.log(pp.T)), consts


def kernel(physical_params, eta, W1, b1, W2, b2):
    from concourse.bass_utils import run_bass_kernel_spmd

    eta = np.ascontiguousarray(eta, dtype=np.float32)
    pT, consts = _host_prep(physical_params, W1, b1, W2, b2)

    if "nc" not in _cache:
        _cache["nc"] = _build_nc()
    nc = _cache["nc"]

    in_maps = []
    for g in range(NCORES):
        m = dict(consts)
        m["eta"] = eta[g * RPC:(g + 1) * RPC]
        m["pT"] = np.ascontiguousarray(np.roll(pT, -g * RPC, axis=1))
        in_maps.append(m)

    res = run_bass_kernel_spmd(nc, in_maps, core_ids=list(range(NCORES)))
    _cache["last_results"] = res
    out = np.concatenate(
        [np.asarray(res.results[g]["dy"]).astype(np.float32)
         for g in range(NCORES)], axis=0)
    return out


# revision 40
# speedup vs baseline: 1.2211x; 1.2211x over previous
"""Trainium2 Bass kernel for nn_CANN_39994735460546.

Reference semantics:
  t    = (physical_params[:, :, None] ** PS_POWERS).reshape(B, 64)
  norm = (t - t.mean()) / t.std(ddof=1)          # global scalar stats
  h    = relu(norm) @ W1.T + b1
  c    = h @ W2.T + b2                            # [B, 5]
  dy[b, j] = sum_k c[b,k] * p_k * eta[b,j]^(p_k - 1),  p = [2,5,8,11,14]

Device strategy (8 NeuronCores, pure data parallel over eta rows; each core
owns 512 rows; stage 1 replicated on every core so no collectives needed):

  Stage 1 (tiny, [B,4] -> per-row poly coefficients):
  - ln(params) computed on host (it is an input-only transform), shipped as
    "pT" [4, B], rolled per core so its own rows come first.
  - t64 = exp(rm.T @ lnp) chunks on PE+ACT; the global stats ride for free
    on the exps' accum_out (a second exp with scale=2 gives sum(t^2)).
  - Coefficients come straight out of one [65,128]x[65,5] matmul per
    128-row block (ones row folds the bias in) -> ctiles [128, 5], f32.

  Stage 2 (the heavy part, dy = eta * P(eta^3) with per-row coefficients),
  spread across ALL engines:
  - ACT: s = fp16(Square(eta)); eh = fp16(eta) (Copy). Square/Copy are
    filler functions present in every activation table set -> ZERO table
    reloads (Ln/Exp-based alternatives pay ~1.3us switches per tile).
  - DVE: u = fp16(s*eh) and the full Horner chain. Since
    scalar_tensor_tensor only has a 1x micro-op, Horner is alternating
    tensor_scalar (4x) / tensor_tensor (2x) steps - 8 cheap 16-bit ops
    beat 5 ops of which 3 run at 1x. Coefficient scalars stay f32
    [128,1], which the perf-mode check permits.
  - No GPSIMD, no mid-chain cross-engine hops: measured on hardware,
    gpsimd elementwise ops pay ~5-20us per-op software overheads, and
    ACT round-trips inside the Horner chain stall the in-order DVE
    queue. (DS below can re-enable a Pool stripe for experiments.)
  - dy stored as bf16 (upcast to f32 on host), halving store traffic.
  - Loads ride the SP HWDGE queue; stores ride ACT, emitted one tile late
    so the ACT queue never stalls waiting on DVE.
"""

import sys
import numpy as np

sys.path.insert(0, "/opt/trn_rl_repo")

B = 4096
L = 4096
NCORES = 8
RPC = B // NCORES          # rows per core = 512
NPT = RPC // 128           # 128-row blocks per core = 4
CT = 4096                  # row width
NDT = 2                    # 128-row blocks per stage-2 double-tile
# DVE Horner stripe width per 4096-col tile. DS=CT disables the GPSIMD
# stripe entirely: measured on hardware, gpsimd elementwise ops carry
# ~5-20us per-op overheads (Q7 software path) that CoreSim's cost model
# does not charge - a 768-col Pool stripe cost 227us/rep vs 62us without.
DS = 4096
UNROLL = 4                 # stage-2 passes per hardware-loop iteration
NTOT = float(B * 64)       # elements in t for the global stats

PS_POWERS = np.array([-5.0, -4.0, -3.0, -2.0, -1.5, -1.0, -0.5, 0.0,
                      0.5, 2.0, 1.0 / 3.0, 3.0, 0.25, 4.0, 0.2, 5.0],
                     dtype=np.float32)
POLY_POWERS = np.array([2.0, 5.0, 8.0, 11.0, 14.0], dtype=np.float32)

_cache = {}


def _build_nc(repeat=1, force_unroll=False):
    import concourse.bass as bass
    import concourse.tile as tile
    from concourse import bacc, mybir

    F32 = mybir.dt.float32
    BF16 = mybir.dt.bfloat16
    F16 = mybir.dt.float16
    AF = mybir.ActivationFunctionType
    OP = mybir.AluOpType
    AX = mybir.AxisListType
    ts = bass.ts

    k1 = 1.0 / (NTOT - 1.0)
    k2 = 1.0 / (NTOT * (NTOT - 1.0))

    nc = bacc.Bacc("TRN2", target_bir_lowering=False, debug=False,
                   num_devices=NCORES)

    eta_d = nc.dram_tensor("eta", [RPC, L], F32, kind="ExternalInput").ap()
    pT_d = nc.dram_tensor("pT", [4, B], F32, kind="ExternalInput").ap()
    rm_d = nc.dram_tensor("rm", [4, 64], F32, kind="ExternalInput").ap()
    wpT_d = nc.dram_tensor("wpT", [65, 5], F32, kind="ExternalInput").ap()
    ones64_d = nc.dram_tensor("ones64", [64, 1], F32, kind="ExternalInput").ap()
    onesr_d = nc.dram_tensor("onesr", [1, 64], F32, kind="ExternalInput").ap()
    dy_d = nc.dram_tensor("dy", [RPC, L], BF16, kind="ExternalOutput").ap()

    from contextlib import ExitStack

    with tile.TileContext(nc) as tc, ExitStack() as stack:
        if True:
            p_const = stack.enter_context(tc.tile_pool(name="consts", bufs=1))
            p_pss = stack.enter_context(
                tc.tile_pool(name="ps_small", bufs=1, space="PSUM"))
            p_psr = stack.enter_context(
                tc.tile_pool(name="ps_r", bufs=2, space="PSUM"))
            p_psc = stack.enter_context(
                tc.tile_pool(name="ps_c", bufs=2, space="PSUM"))
            # ---- constants (rm on SP first: the matmuls wait on it; the
            # rest ride the ACT ring so they don't delay pT/eta) ----------
            rm_sb = p_const.tile([4, 64], F32, tag="rm")
            nc.sync.dma_start(rm_sb[:], rm_d)
            wpT_sb = p_const.tile([65, 5], F32, tag="wpT")
            nc.scalar.dma_start(wpT_sb[:], wpT_d)
            ones64_sb = p_const.tile([64, 1], F32, tag="ones64")
            nc.scalar.dma_start(ones64_sb[:], ones64_d)
            onesr_sb = p_const.tile([1, 64], F32, tag="onesr")
            nc.scalar.dma_start(onesr_sb[:], onesr_d)
            ctiles = [p_const.tile([128, 5], F32, tag=f"ct{t}",
                                   name=f"ct{t}") for t in range(NPT)]

            # Stage-2 pools open BEFORE the stage-1 scratch pool: the stack
            # allocator then gives them disjoint SBUF regions, so the first
            # eta loads don't serialize behind stage-1 reads of recycled
            # addresses.
            p_eta = stack.enter_context(tc.tile_pool(name="eta", bufs=4))
            p_s = stack.enter_context(tc.tile_pool(name="s", bufs=2))
            p_eb = stack.enter_context(tc.tile_pool(name="eb", bufs=2))
            p_u = stack.enter_context(tc.tile_pool(name="u", bufs=2))
            p_g = stack.enter_context(tc.tile_pool(name="g", bufs=2))

            # ---- stage 1 in its own (stack-freed) scratch pool ----
            with (
                tc.tile_pool(name="s1", bufs=1) as p_s1,
                tc.tile_pool(name="s1scr", bufs=2) as p_scr,
            ):
                # pT rides the SP queue FIRST and in 8 chunks: it heads the
                # stage-1 critical path (matmul j waits only on chunk j)
                pT_sb = p_s1.tile([4, B], F32, tag="pT")
                for j in range(B // 512):
                    nc.sync.dma_start(pT_sb[:, ts(j, 512)],
                                      pT_d[:, ts(j, 512)])
                # town rows 0..63: exp chunk for own rows (chunk 0 after the
                # per-core roll); row 64: ones (folds the MLP bias in).
                town = p_s1.tile([65, 512], F32, tag="town")
                acc = p_s1.tile([64, 16], F32, tag="acc")
                sq = p_s1.tile([64, 512], F32, tag="sq")
                nc.vector.memset(town[64:65, :], 1.0)

                for j in range(B // 512):
                    ps_r = p_psr.tile([64, 512], F32, tag="ps_r")
                    nc.tensor.matmul(ps_r[:], rm_sb[:], pT_sb[:, ts(j, 512)],
                                     start=True, stop=True)
                    if j == 0:
                        out_t = town[0:64, :]
                    else:
                        scr_t = p_scr.tile([64, 512], F32, tag="scr",
                                           name=f"scr{j}")
                        out_t = scr_t[:]
                    nc.scalar.activation(out_t, ps_r[:], AF.Exp,
                                         accum_out=acc[:, j:j + 1])
                    # S2 rides DVE (sum of t^2 via accum_out) so the ACT
                    # queue only runs 8 exps, not 16
                    nc.vector.scalar_tensor_tensor(
                        sq[:], out_t, 1.0, out_t, OP.mult, OP.mult,
                        accum_out=acc[:, 8 + j:9 + j])

                # s12[:,0] = sum_j S1 chunks, s12[:,1] = sum_j S2 chunks
                s12 = p_s1.tile([64, 2], F32, tag="s12")
                nc.vector.tensor_reduce(s12[:, 0:1], acc[:, 0:8], AX.X, OP.add)
                nc.vector.tensor_reduce(s12[:, 1:2], acc[:, 8:16], AX.X, OP.add)

                # cross-partition: [1,2] = ones64.T @ s12
                ps_s = p_pss.tile([1, 2], F32, tag="ps_s")
                nc.tensor.matmul(ps_s[:], ones64_sb[:], s12[:],
                                 start=True, stop=True)
                s12sb = p_s1.tile([1, 2], F32, tag="s12sb")
                nc.vector.tensor_copy(s12sb[:], ps_s[:])

                # var = S2/(N-1) - S1^2/(N(N-1)); inv_std = exp(-0.5 ln var)
                scrs = p_s1.tile([1, 4], F32, tag="scrs")
                ab = p_s1.tile([1, 2], F32, tag="ab")
                nc.vector.tensor_scalar(scrs[:, 0:1], s12sb[:, 0:1],
                                        s12sb[:, 0:1], -k2, OP.mult, OP.mult)
                nc.vector.scalar_tensor_tensor(scrs[:, 1:2], s12sb[:, 1:2],
                                               k1, scrs[:, 0:1],
                                               OP.mult, OP.add)
                nc.scalar.activation(scrs[:, 2:3], scrs[:, 1:2], AF.Ln)
                nc.scalar.activation(ab[:, 0:1], scrs[:, 2:3], AF.Exp,
                                     scale=-0.5)
                nc.vector.scalar_tensor_tensor(ab[:, 1:2], s12sb[:, 0:1],
                                               -1.0 / NTOT, ab[:, 0:1],
                                               OP.mult, OP.mult)

                # broadcast (inv_std, bias) to 64 partitions via ones matmul
                ps_b = p_pss.tile([64, 2], F32, tag="ps_b")
                nc.tensor.matmul(ps_b[:], onesr_sb[:], ab[:],
                                 start=True, stop=True)
                ab64 = p_s1.tile([64, 2], F32, tag="ab64")
                nc.vector.tensor_copy(ab64[:], ps_b[:])

                # rn = relu(inv_std * t + bias), in place on town rows 0..63
                nc.scalar.activation(town[0:64, :], town[0:64, :], AF.Relu,
                                     scale=ab64[:, 0:1], bias=ab64[:, 1:2])

                # per 128-row block: ctile [128,5] = town_blk.T @ wpT
                # (row 64 of town is ones -> adds the bias row of wpT)
                for t in range(NPT):
                    ps_c = p_psc.tile([128, 5], F32, tag="ps_c")
                    nc.tensor.matmul(ps_c[:], town[:, ts(t, 128)], wpT_sb[:],
                                     start=True, stop=True)
                    nc.vector.tensor_copy(ctiles[t][:], ps_c[:])

            # ---- stage 2: dy = eta * P(eta^3) ----
            state = {"pending": None}  # store delayed one tile

            def one_pass():
                for t in range(NPT):
                    rows = slice(t * 128, (t + 1) * 128)
                    eta_t = p_eta.tile([128, CT], F32, tag="eta",
                                       name="eta_t")
                    nc.sync.dma_start(eta_t[:], eta_d[rows, :])

                    s_t = p_s.tile([128, CT], F16, tag="s", name="s_t")
                    nc.scalar.activation(s_t[:], eta_t[:], AF.Square)
                    eh_t = p_eb.tile([128, CT], F16, tag="eh", name="eh_t")
                    nc.scalar.activation(eh_t[:], eta_t[:], AF.Copy)
                    if state["pending"] is not None:
                        nc.scalar.dma_start(*state["pending"])
                    u_t = p_u.tile([128, CT], F16, tag="u", name="u_t")
                    g_t = p_g.tile([128, CT], BF16, tag="g", name="g_t")
                    cs = ctiles[t]
                    c0, c1, c2, c3, c4 = (cs[:, k:k + 1] for k in range(5))
                    # DVE stripe [0, DS) and Pool stripe [DS, CT);
                    # each computes its own cube and Horner chain
                    dv = slice(0, DS)
                    g_, u_, eh_ = g_t[:, dv], u_t[:, dv], eh_t[:, dv]
                    nc.vector.tensor_tensor(u_, s_t[:, dv], eh_, OP.mult)
                    nc.vector.tensor_scalar(g_, u_, c4, c3, OP.mult, OP.add)
                    nc.vector.tensor_tensor(g_, g_, u_, OP.mult)
                    nc.vector.tensor_scalar(g_, g_, c2, None, OP.add)
                    nc.vector.tensor_tensor(g_, g_, u_, OP.mult)
                    nc.vector.tensor_scalar(g_, g_, c1, None, OP.add)
                    nc.vector.tensor_tensor(g_, g_, u_, OP.mult)
                    nc.vector.tensor_scalar(g_, g_, c0, None, OP.add)
                    nc.vector.tensor_tensor(g_, g_, eh_, OP.mult)

                    if DS < CT:
                        pl = slice(DS, CT)
                        gp, up, ep = g_t[:, pl], u_t[:, pl], eh_t[:, pl]
                        nc.gpsimd.tensor_tensor(up, s_t[:, pl], ep, OP.mult)
                        nc.gpsimd.tensor_scalar(gp, up, c4, c3,
                                                OP.mult, OP.add)
                        nc.gpsimd.tensor_tensor(gp, gp, up, OP.mult)
                        nc.gpsimd.tensor_scalar(gp, gp, c2, None, OP.add)
                        nc.gpsimd.tensor_tensor(gp, gp, up, OP.mult)
                        nc.gpsimd.tensor_scalar(gp, gp, c1, None, OP.add)
                        nc.gpsimd.tensor_tensor(gp, gp, up, OP.mult)
                        nc.gpsimd.tensor_scalar(gp, gp, c0, None, OP.add)
                        nc.gpsimd.tensor_tensor(gp, gp, ep, OP.mult)
                    state["pending"] = (dy_d[rows, :], g_t[:])

            def flush():
                if state["pending"] is not None:
                    nc.scalar.dma_start(*state["pending"])
                    state["pending"] = None

            if repeat <= UNROLL or force_unroll:
                for _ in range(repeat):
                    one_pass()
                flush()
            else:
                # hardware loop: constant NEFF size for any repeat count, so
                # huge repeats amplify the timing signal above the multi-
                # second axon dispatch noise. UNROLL passes per iteration
                # amortize the per-iteration all-engine barrier.
                n_iter, rem = divmod(repeat, UNROLL)
                with tc.For_i(0, n_iter):
                    for _ in range(UNROLL):
                        one_pass()
                    flush()
                for _ in range(rem):
                    one_pass()
                flush()
    nc.compile()
    return nc


def _host_prep(physical_params, W1, b1, W2, b2):
    pp = np.ascontiguousarray(physical_params, dtype=np.float32)
    W1 = np.asarray(W1, dtype=np.float32)
    b1 = np.asarray(b1, dtype=np.float32)
    W2 = np.asarray(W2, dtype=np.float32)
    b2 = np.asarray(b2, dtype=np.float32)

    # fused MLP (no activation between the linears) + fold p_k
    Weff = W2 @ W1                       # [5, 64]
    beff = W2 @ b1 + b2                  # [5]
    Wp = POLY_POWERS[:, None] * Weff     # [5, 64]
    bp = POLY_POWERS * beff              # [5]

    # [65, 5]: MLP weights with the bias as a final row (ones-row trick)
    wpT = np.concatenate([Wp.T, bp[None, :]], axis=0)

    # replication+scale matrix: rm[i, i*16+j] = PS_POWERS[j]
    rm = np.zeros((4, 64), np.float32)
    for i in range(4):
        rm[i, i * 16:(i + 1) * 16] = PS_POWERS

    consts = {
        "rm": rm,
        "wpT": np.ascontiguousarray(wpT, dtype=np.float32),
        "ones64": np.ones((64, 1), np.float32),
        "onesr": np.ones((1, 64), np.float32),
    }
    # ln on host: pT carries ln(params).T
    return np.ascontiguousarray(np.log(pp.T)), consts


def kernel(physical_params, eta, W1, b1, W2, b2):
    from concourse.bass_utils import run_bass_kernel_spmd

    eta = np.ascontiguousarray(eta, dtype=np.float32)
    pT, consts = _host_prep(physical_params, W1, b1, W2, b2)

    if "nc" not in _cache:
        _cache["nc"] = _build_nc()
    nc = _cache["nc"]

    in_maps = []
    for g in range(NCORES):
        m = dict(consts)
        m["eta"] = eta[g * RPC:(g + 1) * RPC]
        m["pT"] = np.ascontiguousarray(np.roll(pT, -g * RPC, axis=1))
        in_maps.append(m)

    res = run_bass_kernel_spmd(nc, in_maps, core_ids=list(range(NCORES)))
    _cache["last_results"] = res
    out = np.concatenate(
        [np.asarray(res.results[g]["dy"]).astype(np.float32)
         for g in range(NCORES)], axis=0)
    return out
